# revision 7
# baseline (speedup 1.0000x reference)
"""Trainium2 Bass kernel for the siamese-kNN classification head.

Reference computation (B=256, N=2000, D=512, C=100):
    scores[b,n] = sigmoid(sum_d w_d * |a[b,d] - S[n,d]| + kb)
    out[b,c]    = (scores @ L)[b,c] / count_c     (0 where count_c == 0)

Strategy (v2)
-------------
Data-parallel over the batch: core i handles rows 32*i..32*i+32, no
collectives.  |x| = 2 relu(x) - x splits the score into a nonlinear slab
(relu via DVE min / ACT relu, alpha-scaled) plus exact linear corrections:
  - per-b constant kb - w.a_b rides the sigmoid bias (fp32, free)
  - per-n term (w.S)_n enters PSUM as a rank-1 bf16 hi/lo matmul (exact
    to ~2^-17)

Producer assignment (tuned so DVE / ACT / PE finish together; GpSimd was
measured at 28.6us per [128,2000] tensor_scalar -- 40x slower than DVE,
and it starves DVE via the shared SBUF ports, so it stays idle):
  - rows 0..NBF_ROWS-1: all 4 chunks as bf16 DVE slabs (4x [128,500] mm)
  - remaining rows: fp8 alpha=64 pairs, DoubleRow dual-fp8 matmuls at 2x
    PE ingest; ACT takes pair0 (+ a few pair1), DVE takes the rest.
Head: s2t chunk0 DMA'd in quarters / chunk1 in halves on the Sync queue,
a2t+sgnb on the ACT queue, small tensors on the GpSimd queue; first slabs
chase the quarter landings; ACT's activation table is primed by a dummy
activation; PE p-state warms on short dummy matmuls.
Tail: last units emit segment-major so per-segment sigmoids start early;
one XBAR dma_start_transpose ([32,2048] -> [128,16,32]) replaces the 16
PE transposes + copies; 16 bf16 label matmuls, 1/count scale, DMA out on
the DVE queue.
"""

import sys

for _p in ("/opt/trn_rl_repo", "/root/.axon_site/_ro/trn_rl_repo"):
    if _p not in sys.path:
        sys.path.append(_p)

import numpy as np

B, N, D, C = 256, 2000, 512, 100
NP = 2048                  # label rows padded to 16 full chunks
NCORES = 8
BSH = B // NCORES          # 32 batch rows per core
DCH = D // 128             # 4 d-chunks
NSEG = 4                   # PSUM free-dim segments
SEG = N // NSEG            # 500
NLAB = NP // 128           # 16 label chunks
ALPHA = 64.0               # fp8 range pre-scale (exact power of 2)
F8C = SEG                  # fp8 matmul free size per segment

# ---- producer assignment ----
NBF_ROWS = 14              # rows 0..13: all four chunks bf16 on DVE
# ACT fp8 pairs: pair0 of rows 14..31 (18) + pair1 of rows 14..16 (3)
# DVE fp8 pairs: pair1 of rows 17..31 (15)
ACT_PAIRS = [(0, b) for b in range(NBF_ROWS, BSH)] + [(1, b) for b in (14, 15, 16)]
DVE_PAIRS = [(1, b) for b in range(17, BSH)]
N_TAIL_SEGMAJOR = 3        # last units emit segment-major for early sigmoids

_CACHE = {}


def _f8_units():
    """Canonical (prod, pair, b) order; index = dense weight-window slot."""
    return [("dve", p, b) for (p, b) in DVE_PAIRS] + [
        ("act", p, b) for (p, b) in ACT_PAIRS
    ]


NF8 = len(_f8_units())


def _plan():
    """Static schedule: producer instruction order + PE consumption order.

    bf16 items: dict(kind='bfs', ch, b [, pieces]) -- pieces = list of
    (lo, hi) col ranges emitted as separate tensor_scalars (early slabs
    chase partial s2t landings).  fp8: dict(kind='f8p', pair, b, prod).
    PE matmul emission order = estimated completion order.
    """
    # trace-calibrated guesses: s2t piece landings / engine first-instr
    Q0_LAND = [9600.0, 10400.0, 11100.0, 11800.0]     # ch0 quarters
    CH1_LAND = [12700.0, 13500.0]                     # ch1 halves
    CH_LAND_FULL = {2: 15000.0, 3: 16500.0}
    AB_LAND = 8300.0
    DVE_T0, ACT_T0 = 9000.0, 9300.0
    T_BF, T_F8H, T_AH = 755.0, 1240.0, 1860.0
    PIECE_OVH = 90.0

    def land(ch, lo, hi):
        """Earliest time cols [lo:hi) of chunk ch are in SBUF."""
        if ch == 0:
            return Q0_LAND[min(3, (hi - 1) // 500)]
        if ch == 1:
            return CH1_LAND[min(1, (hi - 1) // 1000)]
        return CH_LAND_FULL[ch]

    def interleave(la, lb):
        out, ia, ib = [], 0, 0
        while ia < len(la) or ib < len(lb):
            if ib >= len(lb) or (ia < len(la) and ia * len(lb) <= ib * len(la)):
                out.append(la[ia]); ia += 1
            else:
                out.append(lb[ib]); ib += 1
        return out

    # ---- DVE program ----
    bf_items = []
    for ch in range(DCH):
        for b in range(NBF_ROWS):
            u = dict(kind="bfs", ch=ch, b=b)
            bf_items.append(u)
    # first ch0 slab in quarters (chases the landing DMA), second in halves
    bf_items[0]["pieces"] = [(j * 500, (j + 1) * 500) for j in range(4)]
    bf_items[1]["pieces"] = [(0, 1000), (1000, 2000)]

    f8_dve = [dict(kind="f8p", pair=p, b=b, prod="dve") for (p, b) in DVE_PAIRS]
    f8_act = [dict(kind="f8p", pair=p, b=b, prod="act") for (p, b) in ACT_PAIRS]

    # keep the first 6 DVE slots bf16 (only ch0 data has landed), then
    # interleave the fp8 pairs proportionally
    dve_prog = [("bfs", u) for u in bf_items[:6]]
    for entry in interleave(
        [("bfs", u) for u in bf_items[6:]], [("f8p", u) for u in f8_dve]
    ):
        if entry[0] == "bfs":
            dve_prog.append(entry)
        else:
            dve_prog.append(("f8h", entry[1], 0))
            dve_prog.append(("f8h", entry[1], 1))

    act_prog = []
    for j, u in enumerate(f8_act):
        if j == 0:
            # first unit chases partial chunk landings: both halves split
            for h in (0, 1):
                act_prog.append(("f8hh", u, h, 0))   # [0:1000)
                act_prog.append(("f8hh", u, h, 1))   # [1000:2000)
        else:
            act_prog.append(("f8h", u, 0))
            act_prog.append(("f8h", u, 1))

    def sim(prog, t0):
        clock = t0
        for entry in prog:
            if entry[0] == "bfs":
                u = entry[1]
                for lo, hi in u.get("pieces", [(0, N)]):
                    t = T_BF * (hi - lo) / N + (PIECE_OVH if "pieces" in u else 0.0)
                    clock = max(clock, land(u["ch"], lo, hi), AB_LAND) + t
                u["done"] = clock
            elif entry[0] == "f8h":
                _, u, h = entry
                ch = 2 * u["pair"] + h
                t = T_F8H if u["prod"] == "dve" else T_AH
                clock = max(clock, land(ch, 0, N), AB_LAND) + t
                if h == 1:
                    u["done"] = clock
            else:  # f8hh: ACT half-chunk piece
                _, u, h, piece = entry
                ch = 2 * u["pair"] + h
                lo, hi = (0, 1000) if piece == 0 else (1000, 2000)
                clock = max(clock, land(ch, lo, hi), AB_LAND) + T_AH / 2 + PIECE_OVH
                if h == 1 and piece == 1:
                    u["done"] = clock
        return clock

    sim(dve_prog, DVE_T0)
    sim(act_prog, ACT_T0)

    widx = {(u["prod"], u["pair"], u["b"]): j for j, u in enumerate(
        dict(prod=p, pair=q, b=b) for (p, q, b) in _f8_units())}
    items = bf_items + f8_dve + f8_act
    for u in items:
        if u["kind"] == "f8p":
            u["widx"] = widx[(u["prod"], u["pair"], u["b"])]
    pe_order = sorted(items, key=lambda u: u["done"])
    assert pe_order[0]["kind"] == "bfs" and pe_order[0]["ch"] == 0
    return dve_prog, act_prog, pe_order


def _split_multi_waits(nc):
    """TRN2 TPB instructions encode at most ONE semaphore wait; split extras
    into single-wait NOPs directly before the instruction (same engine)."""
    from concourse import mybir

    for fn in nc.m.functions:
        for bb in fn.blocks:
            out = []
            for inst in bb.instructions:
                si = inst.sync_info
                if si is not None and si.on_wait and len(si.on_wait) > 1:
                    waits = list(si.on_wait)
                    for j, w in enumerate(waits[:-1]):
                        out.append(mybir.InstNoOp(
                            name=f"{inst.name}-sw{j}", engine=inst.engine,
                            sync_info=mybir.SyncInfo(on_wait=[w], on_update=[]),
                            ins=[], outs=[]))
                    inst.sync_info = mybir.SyncInfo(
                        on_wait=[waits[-1]], on_update=list(si.on_update))
                out.append(inst)
            bb.instructions = out


def _build_nc():
    import concourse.bass as bass
    import concourse.tile as tile
    from concourse import mybir

    f32 = mybir.dt.float32
    bf16 = mybir.dt.bfloat16
    f8 = mybir.dt.float8e4
    nc = bass.Bass()

    s2t_d = nc.declare_dram_parameter("s2t", [D, N], bf16, isOutput=False)
    a2t_d = nc.declare_dram_parameter("a2t", [DCH, 128, BSH], f32, isOutput=False)
    sgnb_d = nc.declare_dram_parameter("sgnb", [128, DCH, 63], bf16, isOutput=False)
    sgw_d = nc.declare_dram_parameter("sgw", [128, NF8, 2, 32], f8, isOutput=False)
    wsb_d = nc.declare_dram_parameter("wsb", [2, N + 32], bf16, isOutput=False)
    kbwa_d = nc.declare_dram_parameter("kbwa", [BSH, 1], f32, isOutput=False)
    recb_d = nc.declare_dram_parameter("recb", [BSH, C], f32, isOutput=False)
    labs_d = nc.declare_dram_parameter("labs", [128, NLAB * C], bf16, isOutput=False)
    out_d = nc.declare_dram_parameter("out", [BSH, C], f32, isOutput=True)

    dve_prog, act_prog, pe_order = _plan()

    with tile.TileContext(nc) as tc:
        with (
            tc.tile_pool(name="const", bufs=1) as const,
            tc.tile_pool(name="dslab", bufs=6) as dpool,
            tc.tile_pool(name="vpair", bufs=4) as vpool,
            tc.tile_pool(name="apair", bufs=4) as apool,
            tc.tile_pool(name="bank", bufs=8, space="PSUM") as bankp,
        ):
            # ---- DMAs.  Critical s2t pieces serial on the Sync queue;
            # a2t+sgnb on the ACT queue; small late tensors on GpSimd.
            s2t0 = const.tile([128, N], bf16, name="s2t0", tag="s2t0")
            for j in range(4):
                nc.sync.dma_start(
                    s2t0[:, j * 500 : (j + 1) * 500],
                    s2t_d[0:128, j * 500 : (j + 1) * 500])
            s2t123 = const.tile([128, 3, N], bf16, name="s2t123", tag="s2t123")
            nc.sync.dma_start(s2t123[:, 0, 0:1000], s2t_d[128:256, 0:1000])
            nc.sync.dma_start(s2t123[:, 0, 1000:N], s2t_d[128:256, 1000:N])
            nc.sync.dma_start(s2t123[:, 1, :], s2t_d[256:384, :])
            nc.sync.dma_start(s2t123[:, 2, :], s2t_d[384:512, :])
            labs = const.tile([128, NLAB * C], bf16, name="labs", tag="labs")
            nc.sync.dma_start(labs[:], labs_d[:])

            a2t = const.tile([128, DCH * BSH], f32, name="a2t", tag="a2t")
            nc.scalar.dma_start(
                a2t[:].rearrange("p (c b) -> p c b", c=DCH),
                a2t_d[:].rearrange("c p b -> p c b"),
            )
            sgnb = const.tile([128, DCH, 63], bf16, name="sgnb", tag="sgnb")
            nc.scalar.dma_start(sgnb[:], sgnb_d[:])

            wsb = const.tile([2, N + 32], bf16, name="wsb", tag="wsb")
            nc.gpsimd.dma_start(wsb[:], wsb_d[:])
            kbwa = const.tile([BSH, 1], f32, name="kbwa", tag="kbwa")
            nc.gpsimd.dma_start(kbwa[:], kbwa_d[:])
            recb = const.tile([BSH, C], f32, name="recb", tag="recb")
            nc.gpsimd.dma_start(recb[:], recb_d[:])
            sgw = const.tile([128, NF8, 2, 32], f8, name="sgw", tag="sgw")
            nc.gpsimd.dma_start(sgw[:], sgw_d[:])

            s2t = [s2t0] + [s2t123[:, k, :] for k in range(3)]

            sub_op = mybir.AluOpType.subtract
            min_op = mybir.AluOpType.min
            relu = mybir.ActivationFunctionType.Relu

            psc = [
                bankp.tile([BSH, SEG], f32, name=f"psc{s}", tag="bank")
                for s in range(NSEG)
            ]

            # ---- PE p-state warmup + ACT table prime while DMAs run
            dummy_sb = const.tile([128, 512], bf16, name="dummy", tag="dummy")
            nc.gpsimd.memset(dummy_sb[:], 0.0)
            dummy_ps = bankp.tile([2, 512], f32, name="dummy_ps", tag="bank")
            for _ in range(6):
                nc.tensor.matmul(
                    dummy_ps[:, 0:256], dummy_sb[:, 0:2], dummy_sb[:, 0:256],
                    start=True, stop=True, skip_group_check=True,
                )
            prime = const.tile([1, 8], bf16, name="prime", tag="prime")
            nc.scalar.activation(
                prime[:], dummy_sb[0:1, 0:8],
                mybir.ActivationFunctionType.Sigmoid)

            # ---- producers (DVE / ACT program order from the plan)
            for entry in dve_prog:
                if entry[0] == "bfs":
                    u = entry[1]
                    slab = dpool.tile([128, N], bf16, name="dslab", tag="dslab")
                    col = u["ch"] * BSH + u["b"]
                    for lo, hi in u.get("pieces", [(0, N)]):
                        nc.vector.tensor_scalar(
                            slab[:, lo:hi], s2t[u["ch"]][:, lo:hi],
                            a2t[:, col : col + 1], 0.0, sub_op, min_op,
                        )
                    u["ap"] = slab
                else:
                    _, u, h = entry
                    if h == 0:
                        u["ap"] = vpool.tile([128, NSEG, 2, F8C], f8,
                                             name="vpair", tag="vpair")
                    ch = 2 * u["pair"] + h
                    nc.vector.tensor_scalar(
                        u["ap"][:, :, h, :], s2t[ch],
                        a2t[:, ch * BSH + u["b"] : ch * BSH + u["b"] + 1],
                        0.0, sub_op, min_op,
                    )
            for entry in act_prog:
                if entry[0] == "f8hh":
                    _, u, h, piece = entry
                    if h == 0 and piece == 0:
                        u["ap"] = apool.tile([128, NSEG, 2, F8C], f8,
                                             name="apair", tag="apair")
                    ch = 2 * u["pair"] + h
                    s0, s1 = (0, 2) if piece == 0 else (2, 4)
                    nc.scalar.activation(
                        u["ap"][:, s0:s1, h, :],
                        s2t[ch][:, s0 * SEG : s1 * SEG], relu,
                        bias=a2t[:, ch * BSH + u["b"] : ch * BSH + u["b"] + 1],
                        scale=-1.0,
                    )
                else:
                    _, u, h = entry
                    if h == 0:
                        u["ap"] = apool.tile([128, NSEG, 2, F8C], f8,
                                             name="apair", tag="apair")
                    ch = 2 * u["pair"] + h
                    nc.scalar.activation(
                        u["ap"][:, :, h, :], s2t[ch], relu,
                        bias=a2t[:, ch * BSH + u["b"] : ch * BSH + u["b"] + 1],
                        scale=-1.0,
                    )

            # ---- PE stream in estimated completion order
            def unit_seg_mm(u, s, first, stop):
                b = u["b"]
                if u["kind"] == "bfs":
                    nc.tensor.matmul(
                        psc[s][:], sgnb[:, u["ch"], 31 - b : 63 - b],
                        u["ap"][:, SEG * s : SEG * (s + 1)],
                        start=first, stop=stop,
                        skip_group_check=not first,
                    )
                else:
                    nc.tensor.matmul(
                        psc[s][:], sgw[:, u["widx"], :, :],
                        u["ap"][:, s, :, :],
                        start=False, stop=stop,
                        perf_mode=mybir.MatmulPerfMode.DoubleRow,
                        skip_group_check=True,
                    )

            n_units = len(pe_order)
            head_units = pe_order[: n_units - N_TAIL_SEGMAJOR]
            tail_units = pe_order[n_units - N_TAIL_SEGMAJOR :]
            for idx, u in enumerate(head_units):
                for s in range(NSEG):
                    unit_seg_mm(u, s, first=(idx == 0), stop=False)
                if idx == 2:
                    # exact rank-1 correction: (w.S)_n as bf16 hi+lo rows
                    for s in range(NSEG):
                        nc.tensor.matmul(
                            psc[s][:], wsb[:, N : N + 32],
                            wsb[:, SEG * s : SEG * (s + 1)],
                            start=False, stop=False, skip_group_check=True,
                        )
            # last units segment-major: each segment's accumulation stops as
            # early as possible so its sigmoid can fire
            for s in range(NSEG):
                for j, u in enumerate(tail_units):
                    unit_seg_mm(u, s, first=False,
                                stop=(j == len(tail_units) - 1))

            # ---- sigmoid (PSUM -> SBUF bf16, + per-b bias kb - w.a) ----
            ssig = const.tile([BSH, NP], bf16, name="ssig", tag="ssig")
            for s in range(NSEG):
                nc.scalar.activation(
                    ssig[:, SEG * s : SEG * (s + 1)], psc[s][:],
                    mybir.ActivationFunctionType.Sigmoid, bias=kbwa[:],
                )

            # ---- XBAR transpose [32,2048] -> [128,16,32] on the ACT queue
            sct = const.tile([128, NLAB, BSH], bf16, name="sct", tag="sct")
            nc.scalar.dma_start(sct[:], ssig[:], transpose=True)

            # ---- label matmuls ----
            out_ps = bankp.tile([BSH, C], f32, name="out_ps", tag="bank")
            for k in range(NLAB):
                pk = min(128, N - 128 * k)
                nc.tensor.matmul(
                    out_ps[:], sct[:pk, k, :],
                    labs[:pk, C * k : C * (k + 1)],
                    start=(k == 0), stop=(k == NLAB - 1),
                )

            # ---- divide by counts, write out (ACT queue) ----
            out_s = const.tile([BSH, C], f32, name="out_s", tag="out_s")
            nc.vector.tensor_mul(out_s[:], out_ps[:], recb[:])
            nc.scalar.dma_start(out_d[:], out_s[:])

    _split_multi_waits(nc)
    return nc


def _prep_host(inputs, support_tensors, support_labels, kernel_w, kernel_b):
    import ml_dtypes

    bf16 = ml_dtypes.bfloat16
    f8 = ml_dtypes.float8_e4m3
    a = np.asarray(inputs, dtype=np.float32)
    S = np.asarray(support_tensors, dtype=np.float32)
    L = np.asarray(support_labels, dtype=np.float32)
    w = np.asarray(kernel_w, dtype=np.float32)
    kb = np.float32(np.asarray(kernel_b, dtype=np.float32))

    aw = ALPHA * 2.0 * np.abs(w)
    sgn = np.sign(w).astype(np.float32)
    s2t = np.ascontiguousarray((S * aw[None, :]).T).astype(bf16)   # [D, N]
    wS = (S @ w).astype(np.float32)                                # [N]
    wa = (a @ w).astype(np.float32)                                # [B]
    a2 = a * aw[None, :]                                           # [B, D]

    sgn_chunks = sgn.reshape(DCH, 128).T                           # [128, DCH]
    # bf16 sliding-window sign tiles (negative slabs): col 31 = -sgn/alpha
    sgnb = np.zeros((128, DCH, 63), dtype=np.float32)
    sgnb[:, :, 31] = -sgn_chunks / ALPHA
    # dense fp8 weight windows, one [128,2,32] slot per fp8 unit
    sgw = np.zeros((128, NF8, 2, 32), dtype=np.float32)
    for j, (prod, p, b) in enumerate(_f8_units()):
        pol = -1.0 if prod == "dve" else 1.0
        for i in range(2):
            sgw[:, j, i, b] = pol * sgn_chunks[:, 2 * p + i] / ALPHA
    sgw = sgw.astype(f8)

    # rank-1 correction rows: wS split hi/lo in bf16 (exact to ~2^-17),
    # cols N..N+32 hold the all-ones [2,32] weight window
    ws_hi = wS.astype(bf16)
    ws_lo = (wS - ws_hi.astype(np.float32)).astype(bf16)
    wsb = np.zeros((2, N + 32), dtype=np.float32)
    wsb[0, :N] = ws_hi.astype(np.float32)
    wsb[1, :N] = ws_lo.astype(np.float32)
    wsb[:, N:] = 1.0
    wsb = wsb.astype(bf16)

    labp = np.zeros((NP, C), dtype=np.float32)
    labp[:N] = L
    labs = labp.reshape(NLAB, 128, C).transpose(1, 0, 2).reshape(128, NLAB * C)
    labs = labs.astype(bf16)

    counts = L.sum(axis=0)
    recip = np.where(counts != 0, 1.0 / np.maximum(counts, 1e-30), 0.0)
    recb = np.broadcast_to(recip.astype(np.float32), (BSH, C)).copy()

    shared = {
        "s2t": s2t, "sgnb": sgnb.astype(bf16), "sgw": sgw,
        "wsb": wsb, "labs": labs, "recb": recb,
    }
    in_maps = []
    for c in range(NCORES):
        rows = slice(BSH * c, BSH * (c + 1))
        a2t_c = np.ascontiguousarray(
            a2[rows].T.reshape(DCH, 128, BSH))                     # [DCH,128,BSH]
        kbwa = (kb - wa[rows]).reshape(BSH, 1).astype(np.float32)
        in_maps.append(dict(shared, a2t=a2t_c, kbwa=kbwa))
    return in_maps


def kernel(**inputs) -> np.ndarray:
    from concourse.bass_utils import run_bass_kernel_spmd

    if "nc" not in _CACHE:
        _CACHE["nc"] = _build_nc()
    nc = _CACHE["nc"]

    in_maps = _prep_host(
        inputs["inputs"], inputs["support_tensors"], inputs["support_labels"],
        inputs["kernel_w"], inputs["kernel_b"],
    )
    res = run_bass_kernel_spmd(nc, in_maps, list(range(NCORES)))
    return np.concatenate([res.results[i]["out"] for i in range(NCORES)], axis=0)


# revision 11
# speedup vs baseline: 1.0233x; 1.0233x over previous
"""Trainium2 Bass kernel for the siamese-kNN classification head.

Reference computation (B=256, N=2000, D=512, C=100):
    scores[b,n] = sigmoid(sum_d w_d * |a[b,d] - S[n,d]| + kb)
    out[b,c]    = (scores @ L)[b,c] / count_c     (0 where count_c == 0)

Strategy (v2)
-------------
Data-parallel over the batch: core i handles rows 32*i..32*i+32, no
collectives.  |x| = 2 relu(x) - x splits the score into a nonlinear slab
(relu via DVE min / ACT relu, alpha-scaled) plus exact linear corrections:
  - per-b constant kb - w.a_b rides the sigmoid bias (fp32, free)
  - per-n term (w.S)_n enters PSUM as a rank-1 bf16 hi/lo matmul (exact
    to ~2^-17)

Producer assignment (tuned so DVE / ACT / PE finish together; GpSimd was
measured at 28.6us per [128,2000] tensor_scalar -- 40x slower than DVE,
and it starves DVE via the shared SBUF ports, so it stays idle):
  - rows 0..NBF_ROWS-1: all 4 chunks as bf16 DVE slabs (4x [128,500] mm)
  - remaining rows: fp8 alpha=64 pairs, DoubleRow dual-fp8 matmuls at 2x
    PE ingest; ACT takes pair0 (+ a few pair1), DVE takes the rest.
Head: s2t chunk0 DMA'd in quarters / chunk1 in halves on the Sync queue,
a2t+sgnb on the ACT queue, small tensors on the GpSimd queue; first slabs
chase the quarter landings; ACT's activation table is primed by a dummy
activation; PE p-state warms on short dummy matmuls.
Tail: last units emit segment-major so per-segment sigmoids start early;
one XBAR dma_start_transpose ([32,2048] -> [128,16,32]) replaces the 16
PE transposes + copies; 16 bf16 label matmuls, 1/count scale, DMA out on
the DVE queue.
"""

import sys

for _p in ("/opt/trn_rl_repo", "/root/.axon_site/_ro/trn_rl_repo"):
    if _p not in sys.path:
        sys.path.append(_p)

import numpy as np

B, N, D, C = 256, 2000, 512, 100
NP = 2048                  # label rows padded to 16 full chunks
NCORES = 8
BSH = B // NCORES          # 32 batch rows per core
DCH = D // 128             # 4 d-chunks
NSEG = 4                   # PSUM free-dim segments
SEG = N // NSEG            # 500
NLAB = NP // 128           # 16 label chunks
ALPHA = 64.0               # fp8 range pre-scale (exact power of 2)
F8C = SEG                  # fp8 matmul free size per segment

# ---- producer assignment ----
NBF_ROWS = 14              # rows 0..13: all four chunks bf16 on DVE
# ACT fp8 pairs: pair0 of rows 14..31 (18) + pair1 of rows 14..16 (3)
# DVE fp8 pairs: pair1 of rows 17..31 (15)
ACT_PAIRS = [(0, b) for b in range(NBF_ROWS, BSH)] + [(1, b) for b in (14, 15, 16)]
DVE_PAIRS = [(1, b) for b in range(17, BSH)]
N_TAIL_SEGMAJOR = 3        # last units emit segment-major for early sigmoids

_CACHE = {}


def _f8_units():
    """Canonical (prod, pair, b) order; index = dense weight-window slot."""
    return [("dve", p, b) for (p, b) in DVE_PAIRS] + [
        ("act", p, b) for (p, b) in ACT_PAIRS
    ]


NF8 = len(_f8_units())


def _plan():
    """Static schedule: producer instruction order + PE consumption order.

    bf16 items: dict(kind='bfs', ch, b [, pieces]) -- pieces = list of
    (lo, hi) col ranges emitted as separate tensor_scalars (early slabs
    chase partial s2t landings).  fp8: dict(kind='f8p', pair, b, prod).
    PE matmul emission order = estimated completion order.
    """
    # trace-calibrated guesses: s2t piece landings / engine first-instr
    Q0_LAND = [9600.0, 10400.0, 11100.0, 11800.0]     # ch0 quarters
    CH1_LAND = [12700.0, 13500.0]                     # ch1 halves
    CH_LAND_FULL = {2: 15000.0, 3: 16500.0}
    AB_LAND = 8300.0
    DVE_T0, ACT_T0 = 9000.0, 9300.0
    T_BF, T_F8H, T_AH = 755.0, 1240.0, 1860.0
    PIECE_OVH = 90.0

    def land(ch, lo, hi):
        """Earliest time cols [lo:hi) of chunk ch are in SBUF."""
        if ch == 0:
            return Q0_LAND[min(3, (hi - 1) // 500)]
        if ch == 1:
            return CH1_LAND[min(1, (hi - 1) // 1000)]
        return CH_LAND_FULL[ch]

    def interleave(la, lb):
        out, ia, ib = [], 0, 0
        while ia < len(la) or ib < len(lb):
            if ib >= len(lb) or (ia < len(la) and ia * len(lb) <= ib * len(la)):
                out.append(la[ia]); ia += 1
            else:
                out.append(lb[ib]); ib += 1
        return out

    # ---- DVE program ----
    bf_items = []
    for ch in range(DCH):
        for b in range(NBF_ROWS):
            u = dict(kind="bfs", ch=ch, b=b)
            bf_items.append(u)
    # first ch0 slab in quarters (chases the landing DMA), second in halves
    bf_items[0]["pieces"] = [(j * 500, (j + 1) * 500) for j in range(4)]
    bf_items[1]["pieces"] = [(0, 1000), (1000, 2000)]

    f8_dve = [dict(kind="f8p", pair=p, b=b, prod="dve") for (p, b) in DVE_PAIRS]
    f8_act = [dict(kind="f8p", pair=p, b=b, prod="act") for (p, b) in ACT_PAIRS]

    # keep the first 6 DVE slots bf16 (only ch0 data has landed), then
    # interleave the fp8 pairs proportionally
    dve_prog = [("bfs", u) for u in bf_items[:6]]
    for entry in interleave(
        [("bfs", u) for u in bf_items[6:]], [("f8p", u) for u in f8_dve]
    ):
        if entry[0] == "bfs":
            dve_prog.append(entry)
        else:
            dve_prog.append(("f8h", entry[1], 0))
            dve_prog.append(("f8h", entry[1], 1))

    act_prog = []
    for j, u in enumerate(f8_act):
        if j == 0:
            # first unit chases partial chunk landings: both halves split
            for h in (0, 1):
                act_prog.append(("f8hh", u, h, 0))   # [0:1000)
                act_prog.append(("f8hh", u, h, 1))   # [1000:2000)
        else:
            act_prog.append(("f8h", u, 0))
            act_prog.append(("f8h", u, 1))

    def sim(prog, t0):
        clock = t0
        for entry in prog:
            if entry[0] == "bfs":
                u = entry[1]
                for lo, hi in u.get("pieces", [(0, N)]):
                    t = T_BF * (hi - lo) / N + (PIECE_OVH if "pieces" in u else 0.0)
                    clock = max(clock, land(u["ch"], lo, hi), AB_LAND) + t
                u["done"] = clock
            elif entry[0] == "f8h":
                _, u, h = entry
                ch = 2 * u["pair"] + h
                t = T_F8H if u["prod"] == "dve" else T_AH
                clock = max(clock, land(ch, 0, N), AB_LAND) + t
                if h == 1:
                    u["done"] = clock
            else:  # f8hh: ACT half-chunk piece
                _, u, h, piece = entry
                ch = 2 * u["pair"] + h
                lo, hi = (0, 1000) if piece == 0 else (1000, 2000)
                clock = max(clock, land(ch, lo, hi), AB_LAND) + T_AH / 2 + PIECE_OVH
                if h == 1 and piece == 1:
                    u["done"] = clock
        return clock

    sim(dve_prog, DVE_T0)
    sim(act_prog, ACT_T0)

    widx = {(u["prod"], u["pair"], u["b"]): j for j, u in enumerate(
        dict(prod=p, pair=q, b=b) for (p, q, b) in _f8_units())}
    items = bf_items + f8_dve + f8_act
    for u in items:
        if u["kind"] == "f8p":
            u["widx"] = widx[(u["prod"], u["pair"], u["b"])]
    pe_order = sorted(items, key=lambda u: u["done"])
    assert pe_order[0]["kind"] == "bfs" and pe_order[0]["ch"] == 0
    return dve_prog, act_prog, pe_order


def _split_multi_waits(nc):
    """TRN2 TPB instructions encode at most ONE semaphore wait; split extras
    into single-wait NOPs directly before the instruction (same engine)."""
    from concourse import mybir

    for fn in nc.m.functions:
        for bb in fn.blocks:
            out = []
            for inst in bb.instructions:
                si = inst.sync_info
                if si is not None and si.on_wait and len(si.on_wait) > 1:
                    waits = list(si.on_wait)
                    for j, w in enumerate(waits[:-1]):
                        out.append(mybir.InstNoOp(
                            name=f"{inst.name}-sw{j}", engine=inst.engine,
                            sync_info=mybir.SyncInfo(on_wait=[w], on_update=[]),
                            ins=[], outs=[]))
                    inst.sync_info = mybir.SyncInfo(
                        on_wait=[waits[-1]], on_update=list(si.on_update))
                out.append(inst)
            bb.instructions = out


def _build_nc():
    import concourse.bass as bass
    import concourse.tile as tile
    from concourse import mybir

    f32 = mybir.dt.float32
    bf16 = mybir.dt.bfloat16
    f8 = mybir.dt.float8e4
    nc = bass.Bass()

    s2t_d = nc.declare_dram_parameter("s2t", [D, N], bf16, isOutput=False)
    a2t_d = nc.declare_dram_parameter("a2t", [DCH, 128, BSH], f32, isOutput=False)
    sgnb_d = nc.declare_dram_parameter("sgnb", [128, DCH, 63], bf16, isOutput=False)
    sgw_d = nc.declare_dram_parameter("sgw", [128, NF8, 2, 32], f8, isOutput=False)
    wsb_d = nc.declare_dram_parameter("wsb", [2, N + 32], bf16, isOutput=False)
    kbwa_d = nc.declare_dram_parameter("kbwa", [BSH, 1], f32, isOutput=False)
    recb_d = nc.declare_dram_parameter("recb", [BSH, C], f32, isOutput=False)
    labs_d = nc.declare_dram_parameter("labs", [128, NLAB * C + 32], bf16,
                                       isOutput=False)
    out_d = nc.declare_dram_parameter("out", [BSH, C], f32, isOutput=True)

    dve_prog, act_prog, pe_order = _plan()

    with tile.TileContext(nc) as tc:
        with (
            tc.tile_pool(name="const", bufs=1) as const,
            tc.tile_pool(name="dslab", bufs=6) as dpool,
            tc.tile_pool(name="vpair", bufs=4) as vpool,
            tc.tile_pool(name="apair", bufs=4) as apool,
            tc.tile_pool(name="bank", bufs=8, space="PSUM") as bankp,
        ):
            # ---- DMAs.  Critical s2t pieces serial on the Sync queue;
            # a2t+sgnb on the ACT queue; small late tensors on GpSimd.
            s2t0 = const.tile([128, N], bf16, name="s2t0", tag="s2t0")
            for j in range(4):
                nc.sync.dma_start(
                    s2t0[:, j * 500 : (j + 1) * 500],
                    s2t_d[0:128, j * 500 : (j + 1) * 500])
            s2t123 = const.tile([128, 3, N], bf16, name="s2t123", tag="s2t123")
            nc.sync.dma_start(s2t123[:, 0, 0:1000], s2t_d[128:256, 0:1000])
            nc.sync.dma_start(s2t123[:, 0, 1000:N], s2t_d[128:256, 1000:N])
            nc.sync.dma_start(s2t123[:, 1, :], s2t_d[256:384, :])
            nc.sync.dma_start(s2t123[:, 2, :], s2t_d[384:512, :])
            labs = const.tile([128, NLAB * C + 32], bf16, name="labs", tag="labs")
            nc.sync.dma_start(labs[:], labs_d[:])

            a2t = const.tile([128, DCH * BSH], f32, name="a2t", tag="a2t")
            nc.scalar.dma_start(
                a2t[:].rearrange("p (c b) -> p c b", c=DCH),
                a2t_d[:].rearrange("c p b -> p c b"),
            )
            sgnb = const.tile([128, DCH, 63], bf16, name="sgnb", tag="sgnb")
            nc.scalar.dma_start(sgnb[:], sgnb_d[:])

            wsb = const.tile([2, N + 32], bf16, name="wsb", tag="wsb")
            nc.gpsimd.dma_start(wsb[:], wsb_d[:])
            kbwa = const.tile([BSH, 1], f32, name="kbwa", tag="kbwa")
            nc.gpsimd.dma_start(kbwa[:], kbwa_d[:])
            recb = const.tile([BSH, C], f32, name="recb", tag="recb")
            nc.gpsimd.dma_start(recb[:], recb_d[:])
            sgw = const.tile([128, NF8, 2, 32], f8, name="sgw", tag="sgw")
            nc.gpsimd.dma_start(sgw[:], sgw_d[:])

            s2t = [s2t0] + [s2t123[:, k, :] for k in range(3)]

            sub_op = mybir.AluOpType.subtract
            min_op = mybir.AluOpType.min
            relu = mybir.ActivationFunctionType.Relu

            psc = [
                bankp.tile([BSH, SEG], f32, name=f"psc{s}", tag="bank")
                for s in range(NSEG)
            ]

            # ---- PE p-state warmup + ACT table prime while DMAs run
            dummy_sb = const.tile([128, 512], bf16, name="dummy", tag="dummy")
            nc.gpsimd.memset(dummy_sb[:], 0.0)
            dummy_ps = bankp.tile([2, 512], f32, name="dummy_ps", tag="bank")
            for _ in range(6):
                nc.tensor.matmul(
                    dummy_ps[:, 0:256], dummy_sb[:, 0:2], dummy_sb[:, 0:256],
                    start=True, stop=True, skip_group_check=True,
                )
            prime = const.tile([1, 8], bf16, name="prime", tag="prime")
            nc.scalar.activation(
                prime[:], dummy_sb[0:1, 0:8],
                mybir.ActivationFunctionType.Sigmoid)

            # ---- producers (DVE / ACT program order from the plan)
            for entry in dve_prog:
                if entry[0] == "bfs":
                    u = entry[1]
                    slab = dpool.tile([128, N], bf16, name="dslab", tag="dslab")
                    col = u["ch"] * BSH + u["b"]
                    for lo, hi in u.get("pieces", [(0, N)]):
                        nc.vector.tensor_scalar(
                            slab[:, lo:hi], s2t[u["ch"]][:, lo:hi],
                            a2t[:, col : col + 1], 0.0, sub_op, min_op,
                        )
                    u["ap"] = slab
                else:
                    _, u, h = entry
                    if h == 0:
                        u["ap"] = vpool.tile([128, NSEG, 2, F8C], f8,
                                             name="vpair", tag="vpair")
                    ch = 2 * u["pair"] + h
                    nc.vector.tensor_scalar(
                        u["ap"][:, :, h, :], s2t[ch],
                        a2t[:, ch * BSH + u["b"] : ch * BSH + u["b"] + 1],
                        0.0, sub_op, min_op,
                    )
            for entry in act_prog:
                if entry[0] == "f8hh":
                    _, u, h, piece = entry
                    if h == 0 and piece == 0:
                        u["ap"] = apool.tile([128, NSEG, 2, F8C], f8,
                                             name="apair", tag="apair")
                    ch = 2 * u["pair"] + h
                    s0, s1 = (0, 2) if piece == 0 else (2, 4)
                    nc.scalar.activation(
                        u["ap"][:, s0:s1, h, :],
                        s2t[ch][:, s0 * SEG : s1 * SEG], relu,
                        bias=a2t[:, ch * BSH + u["b"] : ch * BSH + u["b"] + 1],
                        scale=-1.0,
                    )
                else:
                    _, u, h = entry
                    if h == 0:
                        u["ap"] = apool.tile([128, NSEG, 2, F8C], f8,
                                             name="apair", tag="apair")
                    ch = 2 * u["pair"] + h
                    nc.scalar.activation(
                        u["ap"][:, :, h, :], s2t[ch], relu,
                        bias=a2t[:, ch * BSH + u["b"] : ch * BSH + u["b"] + 1],
                        scale=-1.0,
                    )

            # ---- PE stream in estimated completion order
            def unit_seg_mm(u, s, first, stop):
                b = u["b"]
                if u["kind"] == "bfs":
                    nc.tensor.matmul(
                        psc[s][:], sgnb[:, u["ch"], 31 - b : 63 - b],
                        u["ap"][:, SEG * s : SEG * (s + 1)],
                        start=first, stop=stop,
                        skip_group_check=not first,
                    )
                else:
                    nc.tensor.matmul(
                        psc[s][:], sgw[:, u["widx"], :, :],
                        u["ap"][:, s, :, :],
                        start=False, stop=stop,
                        perf_mode=mybir.MatmulPerfMode.DoubleRow,
                        skip_group_check=True,
                    )

            n_units = len(pe_order)
            head_units = pe_order[: n_units - N_TAIL_SEGMAJOR]
            tail_units = pe_order[n_units - N_TAIL_SEGMAJOR :]
            for idx, u in enumerate(head_units):
                for s in range(NSEG):
                    unit_seg_mm(u, s, first=(idx == 0), stop=False)
                if idx == 2:
                    # exact rank-1 correction: (w.S)_n as bf16 hi+lo rows
                    for s in range(NSEG):
                        nc.tensor.matmul(
                            psc[s][:], wsb[:, N : N + 32],
                            wsb[:, SEG * s : SEG * (s + 1)],
                            start=False, stop=False, skip_group_check=True,
                        )
            # last units segment-major: each segment's accumulation stops as
            # early as possible so its sigmoid can fire
            for s in range(NSEG):
                for j, u in enumerate(tail_units):
                    unit_seg_mm(u, s, first=False,
                                stop=(j == len(tail_units) - 1))

            # ---- sigmoid (PSUM -> SBUF bf16, + per-b bias kb - w.a) ----
            ssig = const.tile([BSH, NP], bf16, name="ssig", tag="ssig")
            for s in range(NSEG):
                nc.scalar.activation(
                    ssig[:, SEG * s : SEG * (s + 1)], psc[s][:],
                    mybir.ActivationFunctionType.Sigmoid, bias=kbwa[:],
                )

            # ---- transposes into ONE PSUM bank, copy, label matmuls ----
            ident = labs[0:32, NLAB * C : NLAB * C + 32]
            tpall = bankp.tile([128, NLAB * BSH], bf16, name="tpall", tag="bank")
            sct = const.tile([128, NLAB * BSH], bf16, name="sct", tag="sct")
            out_ps = bankp.tile([BSH, C], f32, name="out_ps", tag="bank")
            for k in range(NLAB):
                pk = min(128, N - 128 * k)
                nc.tensor.transpose(
                    tpall[:pk, BSH * k : BSH * k + BSH],
                    ssig[:, 128 * k : 128 * k + pk], ident,
                )
            nc.vector.tensor_copy(sct[:, : BSH * 8], tpall[:, : BSH * 8])
            nc.vector.tensor_copy(sct[:, BSH * 8 :], tpall[:, BSH * 8 :])
            for k in range(NLAB):
                pk = min(128, N - 128 * k)
                nc.tensor.matmul(
                    out_ps[:], sct[:pk, BSH * k : BSH * k + BSH],
                    labs[:pk, C * k : C * (k + 1)],
                    start=(k == 0), stop=(k == NLAB - 1),
                )

            # ---- divide by counts, write out (ACT queue) ----
            out_s = const.tile([BSH, C], f32, name="out_s", tag="out_s")
            nc.vector.tensor_mul(out_s[:], out_ps[:], recb[:])
            nc.scalar.dma_start(out_d[:], out_s[:])

    _split_multi_waits(nc)
    return nc


def _prep_host(inputs, support_tensors, support_labels, kernel_w, kernel_b):
    import ml_dtypes

    bf16 = ml_dtypes.bfloat16
    f8 = ml_dtypes.float8_e4m3
    a = np.asarray(inputs, dtype=np.float32)
    S = np.asarray(support_tensors, dtype=np.float32)
    L = np.asarray(support_labels, dtype=np.float32)
    w = np.asarray(kernel_w, dtype=np.float32)
    kb = np.float32(np.asarray(kernel_b, dtype=np.float32))

    aw = ALPHA * 2.0 * np.abs(w)
    sgn = np.sign(w).astype(np.float32)
    s2t = np.ascontiguousarray((S * aw[None, :]).T).astype(bf16)   # [D, N]
    wS = (S @ w).astype(np.float32)                                # [N]
    wa = (a @ w).astype(np.float32)                                # [B]
    a2 = a * aw[None, :]                                           # [B, D]

    sgn_chunks = sgn.reshape(DCH, 128).T                           # [128, DCH]
    # bf16 sliding-window sign tiles (negative slabs): col 31 = -sgn/alpha
    sgnb = np.zeros((128, DCH, 63), dtype=np.float32)
    sgnb[:, :, 31] = -sgn_chunks / ALPHA
    # dense fp8 weight windows, one [128,2,32] slot per fp8 unit
    sgw = np.zeros((128, NF8, 2, 32), dtype=np.float32)
    for j, (prod, p, b) in enumerate(_f8_units()):
        pol = -1.0 if prod == "dve" else 1.0
        for i in range(2):
            sgw[:, j, i, b] = pol * sgn_chunks[:, 2 * p + i] / ALPHA
    sgw = sgw.astype(f8)

    # rank-1 correction rows: wS split hi/lo in bf16 (exact to ~2^-17),
    # cols N..N+32 hold the all-ones [2,32] weight window
    ws_hi = wS.astype(bf16)
    ws_lo = (wS - ws_hi.astype(np.float32)).astype(bf16)
    wsb = np.zeros((2, N + 32), dtype=np.float32)
    wsb[0, :N] = ws_hi.astype(np.float32)
    wsb[1, :N] = ws_lo.astype(np.float32)
    wsb[:, N:] = 1.0
    wsb = wsb.astype(bf16)

    labp = np.zeros((NP, C), dtype=np.float32)
    labp[:N] = L
    labsp = labp.reshape(NLAB, 128, C).transpose(1, 0, 2).reshape(128, NLAB * C)
    labs = np.zeros((128, NLAB * C + 32), dtype=np.float32)
    labs[:, : NLAB * C] = labsp
    labs[0:32, NLAB * C :] = np.eye(32, dtype=np.float32)
    labs = labs.astype(bf16)

    counts = L.sum(axis=0)
    recip = np.where(counts != 0, 1.0 / np.maximum(counts, 1e-30), 0.0)
    recb = np.broadcast_to(recip.astype(np.float32), (BSH, C)).copy()

    shared = {
        "s2t": s2t, "sgnb": sgnb.astype(bf16), "sgw": sgw,
        "wsb": wsb, "labs": labs, "recb": recb,
    }
    in_maps = []
    for c in range(NCORES):
        rows = slice(BSH * c, BSH * (c + 1))
        a2t_c = np.ascontiguousarray(
            a2[rows].T.reshape(DCH, 128, BSH))                     # [DCH,128,BSH]
        kbwa = (kb - wa[rows]).reshape(BSH, 1).astype(np.float32)
        in_maps.append(dict(shared, a2t=a2t_c, kbwa=kbwa))
    return in_maps


def kernel(**inputs) -> np.ndarray:
    from concourse.bass_utils import run_bass_kernel_spmd

    if "nc" not in _CACHE:
        _CACHE["nc"] = _build_nc()
    nc = _CACHE["nc"]

    in_maps = _prep_host(
        inputs["inputs"], inputs["support_tensors"], inputs["support_labels"],
        inputs["kernel_w"], inputs["kernel_b"],
    )
    res = run_bass_kernel_spmd(nc, in_maps, list(range(NCORES)))
    return np.concatenate([res.results[i]["out"] for i in range(NCORES)], axis=0)
